# revision 8
# baseline (speedup 1.0000x reference)
"""Complex multi-head attention Trainium2 kernel (8-core SPMD, head-sharded).

Math (per core c, handling global heads 2c and 2c+1, 64 dims each):
  q_r = hidden @ Wq_r^T, q_i = hidden @ Wq_i^T   (hidden is real)
  herm = Q K^H per head; mag = |herm|/8; phase = herm/|herm|
  weights = softmax(mag) with causal mask (no row-max; values are small)
  rot = weights * phase = herm * e * rcp / rowsum(e),
        e = exp(|herm|/8), rcp = 1/|herm|
  attn = rot @ V  (normalization by rowsum deferred past the PV matmul)
  out_partial = attn_c @ Wo_c^T  (channel-sliced) -> summed across cores on host

Layouts: scores are computed transposed ([k, q]) so the PV matmul needs no
transposes; projections produce stacked tiles ([Kr^T;Ki^T] etc.) so each
herm component is a single K=128 matmul. All matmuls run as float32r
(full PE rate at N>=256, fp32-like precision).
"""

import numpy as np

B, S, D = 1, 2048, 1024
H, DH = 16, 64
NCORES = 8
HPC = 2            # heads per core
CPC = HPC * DH     # channels per core = 128
SK = 128           # score k-tile (partition dim)
SQ = 512           # score q-block (free dim)
NK = S // SK       # 16
NQ = S // SQ       # 4
NF = D // 128      # 8 feature chunks


def _build_program():
    import concourse.tile as tile
    from concourse import bacc, mybir

    f32 = mybir.dt.float32
    f32r = mybir.dt.float32r
    AF = mybir.ActivationFunctionType

    nc = bacc.Bacc("TRN2", target_bir_lowering=False)

    hid_t = nc.declare_dram_parameter("hid_t", [D, S], f32r, isOutput=False)
    wq = nc.declare_dram_parameter("wq", [HPC, D, CPC], f32r, isOutput=False)
    wk1 = nc.declare_dram_parameter("wk1", [HPC, D, CPC], f32r, isOutput=False)
    wk2 = nc.declare_dram_parameter("wk2", [HPC, D, CPC], f32r, isOutput=False)
    wv = nc.declare_dram_parameter("wv", [HPC, D, CPC], f32r, isOutput=False)
    wo1 = nc.declare_dram_parameter("wo1", [CPC, 2 * D], f32r, isOutput=False)
    wo2 = nc.declare_dram_parameter("wo2", [CPC, 2 * D], f32r, isOutput=False)
    maskp = nc.declare_dram_parameter("mask01", [128, 128], f32, isOutput=False)
    onesp = nc.declare_dram_parameter("ones", [128, 128], f32r, isOutput=False)
    out = nc.declare_dram_parameter("out", [S, 2 * D], f32, isOutput=True)

    def r(ap):
        return ap

    with tile.TileContext(nc) as tc:
        with tc.tile_pool(name="persist", bufs=1) as persist, \
             tc.tile_pool(name="ps2", bufs=2, space="PSUM") as ps2, \
             tc.tile_pool(name="ps1", bufs=1, space="PSUM") as ps1:

            # ---- persistent SBUF tensors ----
            qs1 = [persist.tile([128, S], f32r, tag=f"qs1_{h}", name=f"qs1_{h}")
                   for h in range(HPC)]
            ks1 = [persist.tile([128, S], f32r, tag=f"ks1_{h}", name=f"ks1_{h}")
                   for h in range(HPC)]
            ks2 = [persist.tile([128, S], f32r, tag=f"ks2_{h}", name=f"ks2_{h}")
                   for h in range(HPC)]
            vs = [persist.tile([128, NK, 128], f32r, tag=f"vs_{h}", name=f"vs_{h}")
                  for h in range(HPC)]
            atr = persist.tile([128, S], f32r, tag="atr")
            ati = persist.tile([128, S], f32r, tag="ati")
            wo1_t = persist.tile([128, 2 * D], f32r, tag="wo1")
            wo2_t = persist.tile([128, 2 * D], f32r, tag="wo2")
            maskt = persist.tile([128, 128], f32, tag="mask")
            onest = persist.tile([128, 128], f32r, tag="ones")
            epst = persist.tile([128, 1], f32, tag="eps")
            nc.vector.memset(epst, 1e-30)

            nc.sync.dma_start(out=wo1_t, in_=wo1[:, :])
            nc.sync.dma_start(out=wo2_t, in_=wo2[:, :])
            nc.sync.dma_start(out=maskt, in_=maskp[:, :])
            nc.sync.dma_start(out=onest, in_=onesp[:, :])

            # ---- phase 1: load hidden^T + weights, run projections ----
            with tc.tile_pool(name="loads", bufs=1) as loads:
                ht = loads.tile([128, NF, S], f32r, tag="ht")
                ht_src = hid_t[:, :].rearrange("(a p) m -> p a m", p=128)
                for j in range(NF):
                    nc.sync.dma_start(out=ht[:, j, :], in_=ht_src[:, j, :])

                wtiles = {}
                for nm, prm in (("wq", wq), ("wk1", wk1), ("wk2", wk2), ("wv", wv)):
                    for h in range(HPC):
                        t = loads.tile([128, NF, CPC], f32r, tag=f"{nm}_{h}",
                                       name=f"{nm}_{h}")
                        src = prm[h, :, :].rearrange("(a p) m -> p a m", p=128)
                        nc.sync.dma_start(out=t, in_=src)
                        wtiles[(nm, h)] = t

                for h in range(HPC):
                    # Q/K projections: out [128 stacked-dims, S]
                    for dest, wname in ((qs1[h], "wq"), (ks1[h], "wk1"),
                                        (ks2[h], "wk2")):
                        wt = wtiles[(wname, h)]
                        for qb in range(NQ):
                            ps = ps2.tile([128, SQ], f32, tag="psA")
                            for j in range(NF):
                                nc.tensor.matmul(
                                    ps, lhsT=r(wt[:, j, :]),
                                    rhs=r(ht[:, j, qb * SQ:(qb + 1) * SQ]),
                                    start=(j == 0), stop=(j == NF - 1))
                            nc.any.tensor_copy(
                                dest[:, qb * SQ:(qb + 1) * SQ], ps)
                    # V projection: out [s-tile partitions, (Vr|Vi)]
                    wt = wtiles[("wv", h)]
                    for st in range(NK):
                        ps = ps2.tile([128, 128], f32, tag="psB")
                        for j in range(NF):
                            nc.tensor.matmul(
                                ps, lhsT=r(ht[:, j, st * 128:(st + 1) * 128]),
                                rhs=r(wt[:, j, :]),
                                start=(j == 0), stop=(j == NF - 1))
                        nc.any.tensor_copy(vs[h][:, st, :], ps)

            # ---- phase 2: attention per head / q-block ----
            with tc.tile_pool(name="work", bufs=2) as work:
                for h in range(HPC):
                    for qb in range(NQ):
                        live_k = 4 * (qb + 1)
                        qsl = slice(qb * SQ, (qb + 1) * SQ)
                        pa = ps1.tile([128, SQ], f32, tag="pva")
                        pb = ps1.tile([128, SQ], f32, tag="pvb")
                        prs = ps1.tile([1, SQ], f32, tag="rs")
                        for kt in range(live_k):
                            ksl = slice(kt * SK, (kt + 1) * SK)
                            diag = kt - 4 * qb  # >=0 inside diagonal 512-group
                            c0 = diag * 128 if diag >= 0 else 0
                            cs = slice(c0, SQ)  # live columns of this tile
                            n_live = SQ - c0

                            hr = ps2.tile([128, SQ], f32, tag="psA")
                            hi = ps2.tile([128, SQ], f32, tag="psB")
                            nc.tensor.matmul(hr[:, cs], lhsT=r(ks1[h][:, ksl]),
                                             rhs=r(qs1[h][:, qsl][:, cs]),
                                             start=True, stop=True)
                            nc.tensor.matmul(hi[:, cs], lhsT=r(ks2[h][:, ksl]),
                                             rhs=r(qs1[h][:, qsl][:, cs]),
                                             start=True, stop=True)

                            sqr = work.tile([128, SQ], f32, tag="sqr")
                            sqi = work.tile([128, SQ], f32, tag="sqi")
                            nc.scalar.activation(sqr[:, cs], hr[:, cs], AF.Square)
                            nc.scalar.activation(sqi[:, cs], hi[:, cs], AF.Square)
                            m2 = work.tile([128, SQ], f32, tag="m2")
                            nc.vector.tensor_add(m2[:, cs], sqr[:, cs], sqi[:, cs])
                            amag = work.tile([128, SQ], f32, tag="amag")
                            nc.scalar.activation(amag[:, cs], m2[:, cs], AF.Sqrt,
                                                 bias=epst[:, :])
                            rcp = work.tile([128, SQ], f32, tag="rcp")
                            nc.vector.reciprocal(rcp[:, cs], amag[:, cs])
                            e = work.tile([128, SQ], f32r, tag="e")
                            nc.scalar.activation(e[:, cs], amag[:, cs], AF.Exp,
                                                 scale=0.125)
                            if diag >= 0:
                                # triangular causal mask on the exact-diagonal
                                # 128x128 subtile (k<=q keeps, k>q zeroed)
                                dsl = slice(c0, c0 + 128)
                                nc.vector.tensor_mul(e[:, dsl], e[:, dsl], maskt)
                            t_ = work.tile([128, SQ], f32, tag="t")
                            nc.vector.tensor_mul(t_[:, cs], e[:, cs], rcp[:, cs])
                            rotr = work.tile([128, SQ], f32r, tag="rotr")
                            roti = work.tile([128, SQ], f32r, tag="roti")
                            nc.vector.tensor_mul(rotr[:, cs], hr[:, cs], t_[:, cs])
                            nc.vector.tensor_mul(roti[:, cs], hi[:, cs], t_[:, cs])

                            first = kt == 0
                            last = kt == live_k - 1
                            nc.tensor.matmul(pa[:, cs], lhsT=r(vs[h][:, kt, :]),
                                             rhs=r(rotr[:, cs]),
                                             start=first, stop=last,
                                             skip_group_check=True)
                            nc.tensor.matmul(pb[:, cs], lhsT=r(vs[h][:, kt, :]),
                                             rhs=r(roti[:, cs]),
                                             start=first, stop=last,
                                             skip_group_check=True)
                            nc.tensor.matmul(prs[:, cs], lhsT=r(onest[:, 0:1]),
                                             rhs=r(e[:, cs]),
                                             start=first, stop=last,
                                             skip_group_check=True)

                        # rowsum -> reciprocal -> broadcast to 128 partitions
                        rsv = work.tile([1, SQ], f32r, tag="rsv")
                        with nc.allow_low_precision("f32r rowsum reciprocal"):
                            nc.vector.reciprocal(rsv, prs)
                        bc = ps1.tile([128, SQ], f32, tag="bc")
                        nc.tensor.matmul(bc, lhsT=r(onest[0:1, :]), rhs=r(rsv),
                                         start=True, stop=True)

                        # combine (complex parts) + normalize -> attnT
                        rsl = slice(h * DH, (h + 1) * DH)
                        par = work.tile([128, SQ], f32, tag="par")
                        nc.scalar.copy(par, pa)  # PSUM -> SBUF
                        cr = work.tile([64, SQ], f32, tag="cr")
                        nc.vector.tensor_sub(cr, par[0:DH, :], pb[DH:2 * DH, :])
                        nc.vector.tensor_mul(atr[rsl, qsl], cr, bc[0:DH, :])
                        ci = work.tile([64, SQ], f32, tag="ci")
                        nc.vector.tensor_add(ci, par[DH:2 * DH, :], pb[0:DH, :])
                        nc.vector.tensor_mul(ati[rsl, qsl], ci, bc[DH:2 * DH, :])

                # ---- phase 3: output projection + store ----
                for qt in range(NK):
                    qsl = slice(qt * 128, (qt + 1) * 128)
                    for nb in range(4):
                        nsl = slice(nb * SQ, (nb + 1) * SQ)
                        po = ps2.tile([128, SQ], f32, tag="psA")
                        nc.tensor.matmul(po, lhsT=r(atr[:, qsl]),
                                         rhs=r(wo1_t[:, nsl]),
                                         start=True, stop=False)
                        nc.tensor.matmul(po, lhsT=r(ati[:, qsl]),
                                         rhs=r(wo2_t[:, nsl]),
                                         start=False, stop=True)
                        ostage = work.tile([128, SQ], f32, tag="ostage",
                                           bufs=3)
                        nc.any.tensor_copy(ostage, po)
                        nc.sync.dma_start(out=out[qsl, nsl], in_=ostage)

    nc.finalize()
    return nc


_NC_CACHE = None


def _get_program():
    global _NC_CACHE
    if _NC_CACHE is None:
        _NC_CACHE = _build_program()
    return _NC_CACHE


def _prep_inputs(hidden, wq_r, wq_i, wk_r, wk_i, wv_r, wv_i, wo_r, wo_i):
    hid_t = np.ascontiguousarray(hidden[0].T)  # [D, S]
    mask01 = np.triu(np.ones((128, 128), dtype=np.float32))
    ones = np.ones((128, 128), dtype=np.float32)

    def cc(a, b):
        return np.ascontiguousarray(np.concatenate([a, b], axis=1))

    in_maps = []
    for c in range(NCORES):
        rows = [slice((2 * c + hl) * DH, (2 * c + hl + 1) * DH)
                for hl in range(HPC)]
        wq_s = np.stack([cc(wq_r[rw].T, wq_i[rw].T) for rw in rows])
        wk1_s = np.stack([cc(wk_r[rw].T, wk_i[rw].T) for rw in rows])
        wk2_s = np.stack([cc(-wk_i[rw].T, wk_r[rw].T) for rw in rows])
        wv_s = np.stack([cc(wv_r[rw].T, wv_i[rw].T) for rw in rows])
        csl = slice(c * CPC, (c + 1) * CPC)
        wo1 = cc(wo_r[:, csl].T, wo_i[:, csl].T)
        wo2 = cc(-wo_i[:, csl].T, wo_r[:, csl].T)
        in_maps.append({
            "hid_t": hid_t, "wq": wq_s, "wk1": wk1_s, "wk2": wk2_s,
            "wv": wv_s, "wo1": wo1, "wo2": wo2,
            "mask01": mask01, "ones": ones,
        })
    return in_maps


def kernel(hidden, wq_r, wq_i, wk_r, wk_i, wv_r, wv_i, wo_r, wo_i,
           _trace=False):
    from concourse.bass_utils import run_bass_kernel_spmd

    args = [np.asarray(a, dtype=np.float32)
            for a in (hidden, wq_r, wq_i, wk_r, wk_i,
                      wv_r, wv_i, wo_r, wo_i)]
    in_maps = _prep_inputs(*args)
    nc = _get_program()
    res = run_bass_kernel_spmd(nc, in_maps, list(range(NCORES)),
                               trace=_trace)
    total = np.zeros((S, 2 * D), dtype=np.float64)
    for c in range(NCORES):
        total += res.results[c]["out"]
    out = total.astype(np.float32)
    full = np.stack([out[:, :D], out[:, D:]], axis=-1)[None]  # [1,S,D,2]
    if _trace:
        return full, res
    return full


# revision 19
# speedup vs baseline: 1.0997x; 1.0997x over previous
"""Complex multi-head attention Trainium2 kernel (8-core SPMD, head-sharded).

Math (per core c, handling global heads 2c and 2c+1, 64 dims each):
  q_r = hidden @ Wq_r^T, q_i = hidden @ Wq_i^T   (hidden is real)
  herm = Q K^H per head; mag = |herm|/8; phase = herm/|herm|
  weights = softmax(mag) with causal mask (no row-max; values are small)
  m2 = hr^2 + hi^2;  l = ln(m2);  a = exp(l/2) = |herm|
  g  = exp((a/4 - l)/2) = exp(a/8)/|herm|;  e = g*a = exp(a/8)
  rot = herm * g;  attn = rot @ V, normalized by rowsum(e) post-PV
  out_partial = attn_c @ Wo_c^T (channel-sliced) -> summed across cores

Layouts: scores are computed transposed ([k, q]) so the PV matmul needs no
transposes; projections produce stacked tiles ([Kr^T;Ki^T] etc.) so each
herm component is a single K=128 matmul. All matmuls run as float32r
(full PE rate at N>=256, fp32-like precision). All ScalarE functions
(Square/Ln/Exp/Copy) live in one LUT set -> a single table load.
"""

import numpy as np

B, S, D = 1, 2048, 1024
H, DH = 16, 64
NCORES = 8
HPC = 2            # heads per core
CPC = HPC * DH     # channels per core = 128
SK = 128           # score k-tile (partition dim)
SQ = 512           # score q-block (free dim)
NK = S // SK       # 16
NQ = S // SQ       # 4
NF = D // 128      # 8 feature chunks


def _build_program():
    import concourse.tile as tile
    from concourse import bacc, bass_isa, mybir

    class OneTableBacc(bacc.Bacc):
        """Every ACT function in this kernel (Square/Ln/Exp/Copy) lives in
        the natural_log_exp_and_others LUT set, so one table load per block
        suffices; the stock pass inserts ~150 redundant reloads (~190us)."""

        def insert_act_table_loads(self):
            from concourse.hw_specs import get_activation_tables
            tables = list(get_activation_tables(self.m.arch).items())
            idx = next(i for i, (k, _) in enumerate(tables)
                       if k == "natural_log_exp_and_others")
            allowed = tables[idx][1]
            used = {i.func for b in self.main_func.blocks
                    for i in b.instructions
                    if isinstance(i, mybir.InstActivation)}
            assert used <= allowed, (used, allowed)
            for blk in self.main_func.blocks:
                for pos, inst in enumerate(blk.instructions):
                    if isinstance(inst, mybir.InstActivation):
                        ld = mybir.InstLoadActFuncSet(
                            name=self.get_next_instruction_name(),
                            ins=[], outs=[], act_func_set_id=idx)
                        ld.engine = inst.engine
                        self.register_instruction(ld)
                        blk.instructions.insert(pos, ld)
                        break

    f32 = mybir.dt.float32
    f32r = mybir.dt.float32r
    AF = mybir.ActivationFunctionType

    nc = OneTableBacc("TRN2", target_bir_lowering=False)

    hid_t = nc.declare_dram_parameter("hid_t", [D, S], f32r, isOutput=False)
    wq = nc.declare_dram_parameter("wq", [HPC, D, CPC], f32r, isOutput=False)
    wk1 = nc.declare_dram_parameter("wk1", [HPC, D, CPC], f32r, isOutput=False)
    wk2 = nc.declare_dram_parameter("wk2", [HPC, D, CPC], f32r, isOutput=False)
    wv = nc.declare_dram_parameter("wv", [HPC, D, CPC], f32r, isOutput=False)
    wo1 = nc.declare_dram_parameter("wo1", [CPC, 2 * D], f32r, isOutput=False)
    wo2 = nc.declare_dram_parameter("wo2", [CPC, 2 * D], f32r, isOutput=False)
    maskp = nc.declare_dram_parameter("mask01", [128, 128], f32, isOutput=False)
    onesp = nc.declare_dram_parameter("ones", [128, 128], f32r, isOutput=False)
    out = nc.declare_dram_parameter("out", [S, 2 * D], f32, isOutput=True)

    def r(ap):
        return ap

    with tile.TileContext(nc) as tc:
        with tc.tile_pool(name="persist", bufs=1) as persist, \
             tc.tile_pool(name="ps2", bufs=2, space="PSUM") as ps2, \
             tc.tile_pool(name="ps1", bufs=2, space="PSUM") as ps1:

            # ---- persistent SBUF tensors ----
            qs1 = [persist.tile([128, S], f32r, tag=f"qs1_{h}", name=f"qs1_{h}")
                   for h in range(HPC)]
            ks1 = [persist.tile([128, S], f32r, tag=f"ks1_{h}", name=f"ks1_{h}")
                   for h in range(HPC)]
            ks2 = [persist.tile([128, S], f32r, tag=f"ks2_{h}", name=f"ks2_{h}")
                   for h in range(HPC)]
            vs = [persist.tile([128, NK, 128], f32r, tag=f"vs_{h}",
                               name=f"vs_{h}")
                  for h in range(HPC)]
            atr = persist.tile([128, S], f32r, tag="atr")
            ati = persist.tile([128, S], f32r, tag="ati")
            wo1_t = persist.tile([128, 2 * D], f32r, tag="wo1")
            wo2_t = persist.tile([128, 2 * D], f32r, tag="wo2")
            maskt = persist.tile([128, 128], f32, tag="mask")
            onest = persist.tile([128, 128], f32r, tag="ones")
            epst = persist.tile([128, 1], f32, tag="eps")
            nc.vector.memset(epst, 1e-30)

            nc.sync.dma_start(out=wo1_t, in_=wo1[:, :])
            nc.sync.dma_start(out=wo2_t, in_=wo2[:, :])
            nc.sync.dma_start(out=maskt, in_=maskp[:, :])
            nc.sync.dma_start(out=onest, in_=onesp[:, :])

            # ---- phase 1: load hidden^T + weights, run projections ----
            with tc.tile_pool(name="loads", bufs=1) as loads:
                ht = loads.tile([128, NF, S], f32r, tag="ht")
                ht_src = hid_t[:, :].rearrange("(a p) m -> p a m", p=128)
                for j in range(NF):
                    nc.sync.dma_start(out=ht[:, j, :], in_=ht_src[:, j, :])

                wtiles = {}
                for nm, prm in (("wq", wq), ("wk1", wk1), ("wk2", wk2),
                                ("wv", wv)):
                    for h in range(HPC):
                        t = loads.tile([128, NF, CPC], f32r, tag=f"{nm}_{h}",
                                       name=f"{nm}_{h}")
                        src = prm[h, :, :].rearrange("(a p) m -> p a m", p=128)
                        nc.sync.dma_start(out=t, in_=src)
                        wtiles[(nm, h)] = t

                for h in range(HPC):
                    # Q/K projections: out [128 stacked-dims, S]
                    for dest, wname in ((qs1[h], "wq"), (ks1[h], "wk1"),
                                        (ks2[h], "wk2")):
                        wt = wtiles[(wname, h)]
                        for qb in range(NQ):
                            ps = ps2.tile([128, SQ], f32, tag="psAB",
                                          name="ps_proj")
                            for j in range(NF):
                                nc.tensor.matmul(
                                    ps, lhsT=r(wt[:, j, :]),
                                    rhs=r(ht[:, j, qb * SQ:(qb + 1) * SQ]),
                                    start=(j == 0), stop=(j == NF - 1))
                            nc.vector.tensor_copy(
                                dest[:, qb * SQ:(qb + 1) * SQ], ps)
                    # V projection: out [s-tile partitions, (Vr|Vi)]
                    wt = wtiles[("wv", h)]
                    for st in range(NK):
                        ps = ps2.tile([128, 128], f32, tag="psAB",
                                      name="ps_vproj")
                        for j in range(NF):
                            nc.tensor.matmul(
                                ps, lhsT=r(ht[:, j, st * 128:(st + 1) * 128]),
                                rhs=r(wt[:, j, :]),
                                start=(j == 0), stop=(j == NF - 1))
                        nc.vector.tensor_copy(vs[h][:, st, :], ps)

            # ---- phase 2+3: attention (heads interleaved per q-block)
            # fused with each finished q-block's output projection ----
            with tc.tile_pool(name="work", bufs=3) as work:
                for qb in range(NQ):
                    qsl = slice(qb * SQ, (qb + 1) * SQ)
                    live_k = 4 * (qb + 1)
                    pabs, eaccs = [], []
                    for h in range(HPC):
                        pabs.append(ps1.tile([128, 2 * SQ], f32, tag="pvab",
                                             name=f"pab{h}"))
                        eaccs.append(work.tile([128, SQ], f32, tag="eacc",
                                               name=f"eacc{h}", bufs=3))

                    def tile_step(h, kt):
                        pab, eacc = pabs[h], eaccs[h]
                        ksl = slice(kt * SK, (kt + 1) * SK)
                        diag = kt - 4 * qb
                        c0 = diag * 128 if diag >= 0 else 0
                        cs = slice(c0, SQ)
                        # herm real | imag in one 2-bank PSUM tile
                        hh = ps2.tile([128, 2 * SQ], f32, tag="psAB",
                                      name="hh")
                        nc.tensor.matmul(hh[:, c0:SQ],
                                         lhsT=r(ks1[h][:, ksl]),
                                         rhs=r(qs1[h][:, qsl][:, cs]),
                                         start=True, stop=True)
                        nc.tensor.matmul(hh[:, SQ + c0:2 * SQ],
                                         lhsT=r(ks2[h][:, ksl]),
                                         rhs=r(qs1[h][:, qsl][:, cs]),
                                         start=True, stop=True)
                        hh2 = hh.rearrange("p (a q) -> p a q", a=2)
                        sq = work.tile([128, 2, SQ], f32, tag="sq", name="sq")
                        nc.scalar.activation(sq[:, :, cs], hh2[:, :, cs],
                                             AF.Square)
                        m2 = work.tile([128, SQ], f32, tag="m2", name="m2",
                                       bufs=4)
                        nc.vector.tensor_add(m2[:, cs], sq[:, 0, cs],
                                             sq[:, 1, cs])
                        lt = work.tile([128, SQ], f32, tag="lt", name="lt",
                                       bufs=4)
                        nc.scalar.activation(lt[:, cs], m2[:, cs], AF.Ln,
                                             bias=epst[:, :])
                        at = work.tile([128, SQ], f32, tag="at", name="at",
                                       bufs=4)
                        nc.scalar.activation(at[:, cs], lt[:, cs], AF.Exp,
                                             scale=0.5)
                        arg = work.tile([128, SQ], f32, tag="arg", name="arg",
                                        bufs=4)
                        nc.vector.scalar_tensor_tensor(
                            arg[:, cs], in0=at[:, cs], scalar=0.25,
                            in1=lt[:, cs],
                            op0=mybir.AluOpType.mult,
                            op1=mybir.AluOpType.subtract)
                        g = work.tile([128, SQ], f32, tag="g", name="g",
                                      bufs=4)
                        nc.scalar.activation(g[:, cs], arg[:, cs], AF.Exp,
                                             scale=0.5)
                        if diag >= 0:
                            dsl = slice(c0, c0 + 128)
                            nc.gpsimd.tensor_mul(g[:, dsl], g[:, dsl], maskt)
                        # eacc (+)= g * at
                        if kt == 0:
                            nc.gpsimd.tensor_mul(eacc, g, at)
                        else:
                            e = work.tile([128, SQ], f32, tag="e", name="e")
                            nc.gpsimd.tensor_mul(e[:, cs], g[:, cs],
                                                 at[:, cs])
                            nc.gpsimd.tensor_add(eacc[:, cs], eacc[:, cs],
                                                 e[:, cs])
                        rot = work.tile([128, 2, SQ], f32r, tag="rot",
                                        name="rot")
                        nc.vector.tensor_mul(rot[:, 0, cs], hh2[:, 0, cs],
                                             g[:, cs])
                        nc.vector.tensor_mul(rot[:, 1, cs], hh2[:, 1, cs],
                                             g[:, cs])
                        first = kt == 0
                        last = kt == live_k - 1
                        nc.tensor.matmul(pab[:, c0:SQ],
                                         lhsT=r(vs[h][:, kt, :]),
                                         rhs=r(rot[:, 0, cs]),
                                         start=first, stop=last,
                                         skip_group_check=True)
                        nc.tensor.matmul(pab[:, SQ + c0:2 * SQ],
                                         lhsT=r(vs[h][:, kt, :]),
                                         rhs=r(rot[:, 1, cs]),
                                         start=first, stop=last,
                                         skip_group_check=True)

                    def block_tail(h):
                        pab, eacc = pabs[h], eaccs[h]
                        # all-partition rowsum (broadcast), then 1/x via
                        # exp(-ln(x)) -- stays in the single ACT LUT set
                        ebc = work.tile([128, SQ], f32, tag="ebc", bufs=2)
                        nc.gpsimd.partition_all_reduce(
                            ebc, eacc, 128, bass_isa.ReduceOp.add)
                        lbc = work.tile([128, SQ], f32, tag="lbc", bufs=2)
                        nc.scalar.activation(lbc, ebc, AF.Ln, bias=epst[:, :])
                        bc = work.tile([128, SQ], f32, tag="bc", bufs=2)
                        nc.scalar.activation(bc, lbc, AF.Exp, scale=-1.0)
                        # combine (complex parts) + normalize -> attnT
                        rsl = slice(h * DH, (h + 1) * DH)
                        par = work.tile([128, SQ], f32, tag="par", bufs=2)
                        nc.scalar.copy(par, pab[:, 0:SQ])  # PSUM -> SBUF
                        cr = work.tile([128, SQ], f32, tag="cr", bufs=2)
                        nc.vector.tensor_sub(cr[rsl, :], par[0:DH, :],
                                             pab[DH:2 * DH, SQ:2 * SQ])
                        nc.vector.tensor_mul(atr[rsl, qsl], cr[rsl, :],
                                             bc[rsl, :])
                        ci = work.tile([128, SQ], f32, tag="ci", bufs=2)
                        nc.vector.tensor_add(ci[rsl, :], par[DH:2 * DH, :],
                                             pab[0:DH, SQ:2 * SQ])
                        nc.vector.tensor_mul(ati[rsl, qsl], ci[rsl, :],
                                             bc[rsl, :])

                    for kt in range(live_k):
                        for h in range(HPC):
                            tile_step(h, kt)
                    for h in range(HPC):
                        block_tail(h)

                    # output projection for this q-block (both heads done);
                    # overlaps the next q-block's attention
                    for qt in range(4 * qb, 4 * qb + 4):
                        tsl = slice(qt * 128, (qt + 1) * 128)
                        for nb in range(4):
                            nsl = slice(nb * SQ, (nb + 1) * SQ)
                            po = ps2.tile([128, SQ], f32, tag="psAB",
                                          name="po")
                            nc.tensor.matmul(po, lhsT=r(atr[:, tsl]),
                                             rhs=r(wo1_t[:, nsl]),
                                             start=True, stop=False)
                            nc.tensor.matmul(po, lhsT=r(ati[:, tsl]),
                                             rhs=r(wo2_t[:, nsl]),
                                             start=False, stop=True)
                            ostage = work.tile([128, SQ], f32, tag="ostage",
                                               bufs=3)
                            nc.vector.tensor_copy(ostage, po)
                            nc.sync.dma_start(out=out[tsl, nsl], in_=ostage)

    nc.finalize()
    return nc


_NC_CACHE = None


def _get_program():
    global _NC_CACHE
    if _NC_CACHE is None:
        _NC_CACHE = _build_program()
    return _NC_CACHE


def _prep_inputs(hidden, wq_r, wq_i, wk_r, wk_i, wv_r, wv_i, wo_r, wo_i):
    hid_t = np.ascontiguousarray(hidden[0].T)  # [D, S]
    mask01 = np.triu(np.ones((128, 128), dtype=np.float32))
    ones = np.ones((128, 128), dtype=np.float32)

    def cc(a, b):
        return np.ascontiguousarray(np.concatenate([a, b], axis=1))

    in_maps = []
    for c in range(NCORES):
        rows = [slice((2 * c + hl) * DH, (2 * c + hl + 1) * DH)
                for hl in range(HPC)]
        wq_s = np.stack([cc(wq_r[rw].T, wq_i[rw].T) for rw in rows])
        wk1_s = np.stack([cc(wk_r[rw].T, wk_i[rw].T) for rw in rows])
        wk2_s = np.stack([cc(-wk_i[rw].T, wk_r[rw].T) for rw in rows])
        wv_s = np.stack([cc(wv_r[rw].T, wv_i[rw].T) for rw in rows])
        csl = slice(c * CPC, (c + 1) * CPC)
        wo1 = cc(wo_r[:, csl].T, wo_i[:, csl].T)
        wo2 = cc(-wo_i[:, csl].T, wo_r[:, csl].T)
        in_maps.append({
            "hid_t": hid_t, "wq": wq_s, "wk1": wk1_s, "wk2": wk2_s,
            "wv": wv_s, "wo1": wo1, "wo2": wo2,
            "mask01": mask01, "ones": ones,
        })
    return in_maps


def kernel(hidden, wq_r, wq_i, wk_r, wk_i, wv_r, wv_i, wo_r, wo_i,
           _trace=False):
    from concourse.bass_utils import run_bass_kernel_spmd

    args = [np.asarray(a, dtype=np.float32)
            for a in (hidden, wq_r, wq_i, wk_r, wk_i,
                      wv_r, wv_i, wo_r, wo_i)]
    in_maps = _prep_inputs(*args)
    nc = _get_program()
    res = run_bass_kernel_spmd(nc, in_maps, list(range(NCORES)),
                               trace=_trace)
    total = np.zeros((S, 2 * D), dtype=np.float64)
    for c in range(NCORES):
        total += res.results[c]["out"]
    out = total.astype(np.float32)
    full = np.stack([out[:, :D], out[:, D:]], axis=-1)[None]  # [1,S,D,2]
    if _trace:
        return full, res
    return full
